# revision 9
# baseline (speedup 1.0000x reference)
"""LoRA first-layer MLP kernel for 8 Trainium2 NeuronCores.

Computation:
    W_eff = W0 + 2.0 * (B @ A)            # [4096, 1024]
    h     = relu(x @ W_eff^T + b0)        # [16384, 4096]
    out   = (h @ W2^T + b2).squeeze(-1)   # [16384]

Sharding: data-parallel over batch; each of the 8 cores handles 2048 rows of
x and replicates the weights. No collectives needed.

Per-core device kernel (all fp32 data, fp32r matmul mode):
  - W0^T streamed to SBUF in [mc2(8), dc(8), 128, 512] blocks; the LoRA
    rank-16 correction 2*(B@A)^T is added into each block on-device (PE
    matmul K=16 -> PSUM, DVE add).
  - Layer 1: h^T[m, b] tiles [128, 512] accumulated on PE over 8 d-chunks
    (lhsT = W_eff^T slice [128d, 128m], rhs = x^T slice [128d, 512b]).
  - relu+bias on ScalarE (bias b0 is per-partition in this layout).
  - Layer 2: out[1, 512b] accumulated on PE over the 32 m-chunks
    (lhsT = W2 chunk [128, 1], rhs = h tile [128, 512]).
"""

import sys

sys.path.insert(0, "/opt/trn_rl_repo")

import numpy as np

import concourse.bacc as bacc
import concourse.bass as bass
import concourse.mybir as mybir
import concourse.tile as tile
from concourse.bass_utils import run_bass_kernel_spmd

F32 = mybir.dt.float32
F32R = mybir.dt.float32r

N_CORES = 8
B_FULL, D, M, R = 16384, 1024, 4096, 16
SCALING = 2.0
BS = B_FULL // N_CORES  # 2048 rows per core
NB = BS // 512  # 4 batch chunks per core
ND = D // 128  # 8 d-chunks
NM = M // 128  # 32 m-chunks
NM2 = M // 512  # 8 m-blocks of 512

_CACHE = {}


def _build_nc():
    nc = bacc.Bacc(
        "TRN2",
        target_bir_lowering=False,
        debug=False,
        num_devices=N_CORES,
    )
    xt = nc.dram_tensor("xt", [NB, 128, ND * 512], F32R, kind="ExternalInput").ap()
    w0t = nc.dram_tensor("w0t", [NM2, ND, 128, 512], F32R, kind="ExternalInput").ap()
    a2t = nc.dram_tensor("a2t", [128, ND * R], F32R, kind="ExternalInput").ap()
    btp = nc.dram_tensor("btp", [R, M], F32R, kind="ExternalInput").ap()
    b0c = nc.dram_tensor("b0c", [128, NM], F32, kind="ExternalInput").ap()
    w2c = nc.dram_tensor("w2c", [128, NM], F32, kind="ExternalInput").ap()
    b2s = nc.dram_tensor("b2s", [1, 1], F32, kind="ExternalInput").ap()
    onesd = nc.dram_tensor("ones", [128, 1], F32, kind="ExternalInput").ap()
    out = nc.dram_tensor("out", [1, BS], F32, kind="ExternalOutput").ap()

    RELU = mybir.ActivationFunctionType.Relu

    with tile.TileContext(nc) as tc:
        with (
            tc.tile_pool(name="wp", bufs=1) as wp,
            tc.tile_pool(name="xp", bufs=2) as xp,
            tc.tile_pool(name="hb", bufs=3) as hb,
            tc.tile_pool(name="cp", bufs=1) as cp,
            tc.tile_pool(name="psh", bufs=3, space="PSUM") as psh,
            tc.tile_pool(name="pso", bufs=2, space="PSUM") as pso,
            tc.tile_pool(name="psl", bufs=2, space="PSUM") as psl,
        ):
            A2T = cp.tile([128, ND * R], F32R, tag="a2t")
            nc.sync.dma_start(out=A2T[:], in_=a2t)
            BT = cp.tile([R, M], F32R, tag="bt")
            nc.sync.dma_start(out=BT[:], in_=btp)
            B0 = cp.tile([128, NM], F32, tag="b0")
            nc.sync.dma_start(out=B0[:], in_=b0c)
            W2 = cp.tile([128, NM], F32, tag="w2")
            nc.sync.dma_start(out=W2[:], in_=w2c)
            B2 = cp.tile([1, 1], F32, tag="b2")
            nc.sync.dma_start(out=B2[:], in_=b2s)
            OS = cp.tile([1, BS], F32, tag="os")
            ONES = cp.tile([128, 1], F32, tag="ones")
            nc.sync.dma_start(out=ONES[:], in_=onesd)

            # Resident W_eff^T, laid out [mc2, dc, 512] along the free dim.
            W = wp.tile([128, NM2 * ND * 512], F32R, tag="w")

            # First x chunk early so PE can start as soon as W blocks land.
            xb0 = xp.tile([128, ND * 512], F32R, tag="xb")
            nc.sync.dma_start(out=xb0[:], in_=xt[0])

            # Stream W0^T; the LoRA term is applied per-tile via U below.
            for mc2 in range(NM2):
                for dc in range(ND):
                    blk = (mc2 * ND + dc) * 512
                    nc.sync.dma_start(out=W[:, blk : blk + 512], in_=w0t[mc2, dc])

            MULT = mybir.AluOpType.mult
            ADD = mybir.AluOpType.add

            for bc in range(NB):
                if bc == 0:
                    xb = xb0
                else:
                    xb = xp.tile([128, ND * 512], F32R, tag="xb")
                    nc.sync.dma_start(out=xb[:], in_=xt[bc])
                # U = (2A) @ x_chunk^T  -> [R, 512], then to SBUF as f32r
                up = psl.tile([R, 512], F32, tag="up")
                for dc in range(ND):
                    nc.tensor.matmul(
                        up[:],
                        A2T[:, dc * R : (dc + 1) * R],
                        xb[:, dc * 512 : (dc + 1) * 512],
                        start=(dc == 0),
                        stop=(dc == ND - 1),
                    )
                U = hb.tile([R, 512], F32R, tag="u")
                nc.vector.tensor_copy(U[:], up[:])
                acc = hb.tile([128, 512], F32, tag="acc")
                for mc in range(NM):
                    mc2, j0 = mc // 4, (mc % 4) * 128
                    hp = psh.tile([128, 512], F32, tag="hp")
                    for dc in range(ND):
                        blk = (mc2 * ND + dc) * 512 + j0
                        nc.tensor.matmul(
                            hp[:],
                            W[:, blk : blk + 128],
                            xb[:, dc * 512 : (dc + 1) * 512],
                            start=(dc == 0),
                            stop=False,
                        )
                    nc.tensor.matmul(
                        hp[:],
                        BT[:, mc * 128 : (mc + 1) * 128],
                        U[:],
                        start=False,
                        stop=True,
                    )
                    h = hb.tile([128, 512], F32, tag="h")
                    nc.scalar.activation(h[:], hp[:], RELU, bias=B0[:, mc : mc + 1])
                    # acc += h * W2[m]  (layer 2, on VectorE)
                    if mc == 0:
                        nc.vector.tensor_scalar_mul(acc[:], h[:], W2[:, mc : mc + 1])
                    else:
                        nc.vector.scalar_tensor_tensor(
                            acc[:], h[:], W2[:, mc : mc + 1], acc[:], MULT, ADD
                        )
                # partition-reduce acc -> [1, 512] on PE, then + b2
                op = pso.tile([1, 512], F32, tag="op")
                nc.tensor.matmul(op[:], ONES[:], acc[:], start=True, stop=True)
                nc.vector.tensor_scalar_add(
                    OS[:, bc * 512 : (bc + 1) * 512], op[:], B2[:, 0:1]
                )
            nc.sync.dma_start(out=out, in_=OS[:])

    nc.compile()
    return nc


def _prep_in_maps(x, W0, b0, A, B, W2, b2):
    w0t_full = np.ascontiguousarray(W0.T).reshape(ND, 128, M)
    # -> [mc2, dc, 128, 512]
    w0t = np.ascontiguousarray(
        w0t_full.reshape(ND, 128, NM2, 512).transpose(2, 0, 1, 3)
    )
    a2t = np.ascontiguousarray(
        (SCALING * A).T.reshape(ND, 128, R).transpose(1, 0, 2).reshape(128, ND * R)
    )
    btp = np.ascontiguousarray(B.T)
    b0c = np.ascontiguousarray(b0.reshape(NM, 128).T)
    w2c = np.ascontiguousarray(W2[0].reshape(NM, 128).T)
    b2s = b2.reshape(1, 1).astype(np.float32)
    ones = np.ones((128, 1), dtype=np.float32)

    in_maps = []
    for c in range(N_CORES):
        xs = x[c * BS : (c + 1) * BS]  # [2048, 1024]
        # xt[bc, p, dc*512 + b] = xs[bc*512 + b, dc*128 + p]
        xt = np.ascontiguousarray(
            xs.reshape(NB, 512, ND, 128).transpose(0, 3, 2, 1).reshape(NB, 128, ND * 512)
        )
        in_maps.append(
            {
                "xt": xt,
                "w0t": w0t,
                "a2t": a2t,
                "btp": btp,
                "b0c": b0c,
                "w2c": w2c,
                "b2s": b2s,
                "ones": ones,
            }
        )
    return in_maps


def kernel(x, W0, b0, A, B, W2, b2, _trace=False, _trace_kwargs=None):
    x = np.asarray(x, dtype=np.float32)
    W0 = np.asarray(W0, dtype=np.float32)
    b0 = np.asarray(b0, dtype=np.float32)
    A = np.asarray(A, dtype=np.float32)
    B = np.asarray(B, dtype=np.float32)
    W2 = np.asarray(W2, dtype=np.float32)
    b2 = np.asarray(b2, dtype=np.float32)

    if "nc" not in _CACHE:
        _CACHE["nc"] = _build_nc()
    nc = _CACHE["nc"]

    in_maps = _prep_in_maps(x, W0, b0, A, B, W2, b2)
    res = run_bass_kernel_spmd(
        nc,
        in_maps,
        list(range(N_CORES)),
        trace=_trace,
        **(_trace_kwargs or {}),
    )
    out = np.concatenate([r["out"].reshape(BS) for r in res.results])
    if _trace:
        _CACHE["last_results"] = res
    return out.astype(np.float32)


# revision 10
# speedup vs baseline: 1.0518x; 1.0518x over previous
"""LoRA first-layer MLP kernel for 8 Trainium2 NeuronCores.

Computation:
    W_eff = W0 + 2.0 * (B @ A)            # [4096, 1024]
    h     = relu(x @ W_eff^T + b0)        # [16384, 4096]
    out   = (h @ W2^T + b2).squeeze(-1)   # [16384]

Sharding: data-parallel over batch; each of the 8 cores handles 2048 rows of
x and replicates the weights. No collectives needed.

Per-core device kernel (all fp32 data, fp32r matmul mode):
  - W0^T streamed to SBUF in [mc2(8), dc(8), 128, 512] blocks; the LoRA
    rank-16 correction 2*(B@A)^T is added into each block on-device (PE
    matmul K=16 -> PSUM, DVE add).
  - Layer 1: h^T[m, b] tiles [128, 512] accumulated on PE over 8 d-chunks
    (lhsT = W_eff^T slice [128d, 128m], rhs = x^T slice [128d, 512b]).
  - relu+bias on ScalarE (bias b0 is per-partition in this layout).
  - Layer 2: out[1, 512b] accumulated on PE over the 32 m-chunks
    (lhsT = W2 chunk [128, 1], rhs = h tile [128, 512]).
"""

import sys

sys.path.insert(0, "/opt/trn_rl_repo")

import numpy as np

import concourse.bacc as bacc
import concourse.bass as bass
import concourse.mybir as mybir
import concourse.tile as tile
from concourse.bass_utils import run_bass_kernel_spmd

F32 = mybir.dt.float32
F32R = mybir.dt.float32r

N_CORES = 8
B_FULL, D, M, R = 16384, 1024, 4096, 16
SCALING = 2.0
BS = B_FULL // N_CORES  # 2048 rows per core
NB = BS // 512  # 4 batch chunks per core
ND = D // 128  # 8 d-chunks
NM = M // 128  # 32 m-chunks
NM2 = M // 512  # 8 m-blocks of 512

_CACHE = {}


def _build_nc():
    nc = bacc.Bacc(
        "TRN2",
        target_bir_lowering=False,
        debug=False,
        num_devices=N_CORES,
    )
    xt = nc.dram_tensor("xt", [NB, 128, ND * 512], F32R, kind="ExternalInput").ap()
    w0t = nc.dram_tensor("w0t", [NM2, ND, 128, 512], F32R, kind="ExternalInput").ap()
    a2t = nc.dram_tensor("a2t", [128, ND * R], F32R, kind="ExternalInput").ap()
    btp = nc.dram_tensor("btp", [128, M], F32R, kind="ExternalInput").ap()
    uzd = nc.dram_tensor("uz", [128 - R, 512], F32R, kind="ExternalInput").ap()
    b0c = nc.dram_tensor("b0c", [128, NM], F32, kind="ExternalInput").ap()
    w2c = nc.dram_tensor("w2c", [128, NM], F32, kind="ExternalInput").ap()
    b2s = nc.dram_tensor("b2s", [1, 1], F32, kind="ExternalInput").ap()
    onesd = nc.dram_tensor("ones", [128, 1], F32, kind="ExternalInput").ap()
    out = nc.dram_tensor("out", [1, BS], F32, kind="ExternalOutput").ap()

    RELU = mybir.ActivationFunctionType.Relu

    with tile.TileContext(nc) as tc:
        with (
            tc.tile_pool(name="wp", bufs=1) as wp,
            tc.tile_pool(name="xp", bufs=2) as xp,
            tc.tile_pool(name="hb", bufs=3) as hb,
            tc.tile_pool(name="cp", bufs=1) as cp,
            tc.tile_pool(name="psh", bufs=3, space="PSUM") as psh,
            tc.tile_pool(name="pso", bufs=2, space="PSUM") as pso,
            tc.tile_pool(name="psl", bufs=2, space="PSUM") as psl,
        ):
            A2T = cp.tile([128, ND * R], F32R, tag="a2t")
            nc.sync.dma_start(out=A2T[:], in_=a2t)
            BT = cp.tile([128, M], F32R, tag="bt")
            nc.sync.dma_start(out=BT[:], in_=btp)
            U = cp.tile([128, 512], F32R, tag="u")
            nc.sync.dma_start(out=U[R:128, :], in_=uzd)
            B0 = cp.tile([128, NM], F32, tag="b0")
            nc.sync.dma_start(out=B0[:], in_=b0c)
            W2 = cp.tile([128, NM], F32, tag="w2")
            nc.sync.dma_start(out=W2[:], in_=w2c)
            B2 = cp.tile([1, 1], F32, tag="b2")
            nc.sync.dma_start(out=B2[:], in_=b2s)
            OS = cp.tile([1, BS], F32, tag="os")
            ONES = cp.tile([128, 1], F32, tag="ones")
            nc.sync.dma_start(out=ONES[:], in_=onesd)

            # Resident W_eff^T, laid out [mc2, dc, 512] along the free dim.
            W = wp.tile([128, NM2 * ND * 512], F32R, tag="w")

            # First x chunk early (per-dc pieces) so PE can start ASAP.
            xb0 = xp.tile([128, ND * 512], F32R, tag="xb")
            for dc in range(ND):
                nc.sync.dma_start(
                    out=xb0[:, dc * 512 : (dc + 1) * 512],
                    in_=xt[0][:, dc * 512 : (dc + 1) * 512],
                )

            # Stream W0^T; the LoRA term is applied per-tile via U below.
            for mc2 in range(NM2):
                for dc in range(ND):
                    blk = (mc2 * ND + dc) * 512
                    nc.sync.dma_start(out=W[:, blk : blk + 512], in_=w0t[mc2, dc])

            MULT = mybir.AluOpType.mult
            ADD = mybir.AluOpType.add

            for bc in range(NB):
                if bc == 0:
                    xb = xb0
                else:
                    xb = xp.tile([128, ND * 512], F32R, tag="xb")
                    for dc in range(ND):
                        nc.sync.dma_start(
                            out=xb[:, dc * 512 : (dc + 1) * 512],
                            in_=xt[bc][:, dc * 512 : (dc + 1) * 512],
                        )
                # U = (2A) @ x_chunk^T  -> [R, 512], then to SBUF as f32r
                up = psl.tile([R, 512], F32, tag="up")
                for dc in range(ND):
                    nc.tensor.matmul(
                        up[:],
                        A2T[:, dc * R : (dc + 1) * R],
                        xb[:, dc * 512 : (dc + 1) * 512],
                        start=(dc == 0),
                        stop=(dc == ND - 1),
                    )
                nc.vector.tensor_copy(U[0:R, :], up[:])
                acc = hb.tile([128, 512], F32, tag="acc")
                for mc in range(NM):
                    mc2, j0 = mc // 4, (mc % 4) * 128
                    hp = psh.tile([128, 512], F32, tag="hp")
                    for dc in range(ND):
                        blk = (mc2 * ND + dc) * 512 + j0
                        nc.tensor.matmul(
                            hp[:],
                            W[:, blk : blk + 128],
                            xb[:, dc * 512 : (dc + 1) * 512],
                            start=(dc == 0),
                            stop=False,
                        )
                    nc.tensor.matmul(
                        hp[:],
                        BT[:, mc * 128 : (mc + 1) * 128],
                        U[:, :],
                        start=False,
                        stop=True,
                    )
                    h = hb.tile([128, 512], F32, tag="h")
                    nc.scalar.activation(h[:], hp[:], RELU, bias=B0[:, mc : mc + 1])
                    # acc += h * W2[m]  (layer 2, on VectorE)
                    if mc == 0:
                        nc.vector.tensor_scalar_mul(acc[:], h[:], W2[:, mc : mc + 1])
                    else:
                        nc.vector.scalar_tensor_tensor(
                            acc[:], h[:], W2[:, mc : mc + 1], acc[:], MULT, ADD
                        )
                # partition-reduce acc -> [1, 512] on PE, then + b2
                op = pso.tile([1, 512], F32, tag="op")
                nc.tensor.matmul(op[:], ONES[:], acc[:], start=True, stop=True)
                nc.vector.tensor_scalar_add(
                    OS[:, bc * 512 : (bc + 1) * 512], op[:], B2[:, 0:1]
                )
            nc.sync.dma_start(out=out, in_=OS[:])

    nc.compile()
    return nc


def _prep_in_maps(x, W0, b0, A, B, W2, b2):
    w0t_full = np.ascontiguousarray(W0.T).reshape(ND, 128, M)
    # -> [mc2, dc, 128, 512]
    w0t = np.ascontiguousarray(
        w0t_full.reshape(ND, 128, NM2, 512).transpose(2, 0, 1, 3)
    )
    a2t = np.ascontiguousarray(
        (SCALING * A).T.reshape(ND, 128, R).transpose(1, 0, 2).reshape(128, ND * R)
    )
    btp = np.zeros((128, M), dtype=np.float32)
    btp[:R] = B.T
    uz = np.zeros((128 - R, 512), dtype=np.float32)
    b0c = np.ascontiguousarray(b0.reshape(NM, 128).T)
    w2c = np.ascontiguousarray(W2[0].reshape(NM, 128).T)
    b2s = b2.reshape(1, 1).astype(np.float32)
    ones = np.ones((128, 1), dtype=np.float32)

    in_maps = []
    for c in range(N_CORES):
        xs = x[c * BS : (c + 1) * BS]  # [2048, 1024]
        # xt[bc, p, dc*512 + b] = xs[bc*512 + b, dc*128 + p]
        xt = np.ascontiguousarray(
            xs.reshape(NB, 512, ND, 128).transpose(0, 3, 2, 1).reshape(NB, 128, ND * 512)
        )
        in_maps.append(
            {
                "xt": xt,
                "w0t": w0t,
                "a2t": a2t,
                "btp": btp,
                "uz": uz,
                "b0c": b0c,
                "w2c": w2c,
                "b2s": b2s,
                "ones": ones,
            }
        )
    return in_maps


def kernel(x, W0, b0, A, B, W2, b2, _trace=False, _trace_kwargs=None):
    x = np.asarray(x, dtype=np.float32)
    W0 = np.asarray(W0, dtype=np.float32)
    b0 = np.asarray(b0, dtype=np.float32)
    A = np.asarray(A, dtype=np.float32)
    B = np.asarray(B, dtype=np.float32)
    W2 = np.asarray(W2, dtype=np.float32)
    b2 = np.asarray(b2, dtype=np.float32)

    if "nc" not in _CACHE:
        _CACHE["nc"] = _build_nc()
    nc = _CACHE["nc"]

    in_maps = _prep_in_maps(x, W0, b0, A, B, W2, b2)
    res = run_bass_kernel_spmd(
        nc,
        in_maps,
        list(range(N_CORES)),
        trace=_trace,
        **(_trace_kwargs or {}),
    )
    out = np.concatenate([r["out"].reshape(BS) for r in res.results])
    if _trace:
        _CACHE["last_results"] = res
    return out.astype(np.float32)
